# revision 33
# baseline (speedup 1.0000x reference)
"""GroupSparseAE (FISTA group-lasso encoder + linear decoder) on 8 trn2 cores.

Sharding (channel-major, 1536 (row, channel) atoms over 8 cores):
  sub A: cores 0-3 take channel 0 rows [128i, 128i+128); cores 4-7 take
         channel 1 rows likewise  -> 128-row block, one W.
  sub B: every core takes channel 2 rows [64i, 64i+64) -> 64-row block.
A's 128-wide moving operand halves the LDWEIGHTS-per-row cost vs 64-wide.

A and B are independent FISTA problems; their per-iteration pipelines are
interleaved (A-matmuls, B-matmuls, A-activation, B-activation) so each
sub's serial activation chain (square -> group-sum matmul -> sqrt -> recip
-> scale -> threshold -> momentum) hides under the other sub's matmuls.

Per sub, per iteration k = 1..30 with state in transposed [D, b] layout:
    u^T    = W^T-contract:  uT[n,b]   = sum_d W[d,n] xT[d,b]
    grad^T = gT[e,b]        = sum_n WT[n,e] uT[n,b]
    v      = xT_tmp + y2 - TAU*gT          (y2 = TAU * W @ x^T, precomputed)
    group soft-threshold (groups of 8 along d = partition dim):
       gs = Bmat^T @ v^2  (Bmat block-diag ones -> broadcast group sumsq)
       xnew = relu(v) * relu(1 - c/sqrt(gs))
    momentum: xtmp = xnew + m_k (xnew - xold)
  decode: out^T[n,b] = sum_d W[d,n] z[d,b]

Precision: all matmul operands bf16 (fp32 matmul = 2 half-speed passes);
the FISTA state (xnew/xold/pre) stays fp32 so quantization does not
accumulate over 30 iterations (numpy sim: 4.6e-3 rel err vs 1.7e-2 with
bf16 state). y2 is stored bf16 (constant perturbation only).

PSUM accumulators are [128, 512] (one bank, 8/4 interleaved chains); only
the first matmul into a bank uses start=True, because first_mm clears the
has_written bits of the WHOLE bank.
"""

import sys

sys.path.insert(0, "/opt/trn_rl_repo")

import numpy as np

B, C, N = 512, 3, 1024
G, S = 256, 8
D = G * S  # 2048
NUM_LAYERS = 30
TAU, LAM = 0.1, 0.1
CTH = LAM * TAU  # group threshold constant

N_CORES = 8
NT = D // 128  # 16 d-tiles
NS = N // 128  # 8 n-tiles
BLA = 128  # sub-A rows per core
BLB = 64  # sub-B rows per core
CHUNK = 512  # elementwise chunk, one PSUM bank of fp32


def _mom_coeffs(num_layers):
    # fp32 t-sequence to match the reference's on-device arithmetic
    one, four, two = np.float32(1.0), np.float32(4.0), np.float32(2.0)
    t = np.float32(1.0)
    ms = []
    for _ in range(num_layers):
        t_new = (one + np.sqrt(one + four * t * t)) / two
        ms.append(float((t - one) / t_new))
        t = t_new
    return ms


def _bmat_np():
    p = np.arange(128)
    return (p[:, None] // S == p[None, :] // S).astype(np.float32)


def build(num_layers=NUM_LAYERS):
    import concourse.bacc as bacc
    from concourse import mybir
    from concourse.tile import TileContext

    fp32 = mybir.dt.float32
    bf16 = mybir.dt.bfloat16
    AF = mybir.ActivationFunctionType
    OP = mybir.AluOpType

    nc = bacc.Bacc("TRN2", target_bir_lowering=False, debug=False,
                   num_devices=N_CORES)
    xta = nc.dram_tensor("xta", [N, BLA], bf16, kind="ExternalInput")
    wa = nc.dram_tensor("wa", [D, N], bf16, kind="ExternalInput")
    wta = nc.dram_tensor("wta", [N, D], bf16, kind="ExternalInput")
    xtb = nc.dram_tensor("xtb", [N, BLB], bf16, kind="ExternalInput")
    wb = nc.dram_tensor("wb", [D, N], bf16, kind="ExternalInput")
    wtb = nc.dram_tensor("wtb", [N, D], bf16, kind="ExternalInput")
    bm = nc.dram_tensor("bm", [128, 128], bf16, kind="ExternalInput")
    ota = nc.dram_tensor("ota", [N, BLA], fp32, kind="ExternalOutput")
    otb = nc.dram_tensor("otb", [N, BLB], fp32, kind="ExternalOutput")

    ms = _mom_coeffs(num_layers)

    with TileContext(nc) as tc:
        with (
            tc.tile_pool(name="wp", bufs=1) as wp,
            tc.tile_pool(name="st", bufs=1) as st,
            tc.tile_pool(name="scr", bufs=2) as scr,
            tc.tile_pool(name="scr1", bufs=2) as scr1,
            tc.tile_pool(name="scr3", bufs=3) as scr3,
            tc.tile_pool(name="ps_u", bufs=3, space="PSUM") as ps_u,
            tc.tile_pool(name="ps_g", bufs=3, space="PSUM") as ps_g,
            tc.tile_pool(name="ps_s", bufs=2, space="PSUM") as ps_s,
        ):
            bmat = wp.tile([128, 128], bf16, tag="bmat")
            eps = wp.tile([128, 1], fp32, tag="eps")
            nc.vector.memset(eps, 1e-30)

            class Sub:
                def __init__(self, q, bl, xt_d, w_d, wt_d, ot_d):
                    self.q = q
                    self.bl = bl
                    self.fd = NT * bl
                    self.nch = self.fd // CHUNK
                    self.tpc = CHUNK // bl  # d-tiles per chunk
                    self.sgp = CHUNK // bl  # s-tiles per pu bank
                    self.npu = NS // self.sgp
                    self.ot_d = ot_d
                    self.wsb = wp.tile([128, NT, N], bf16, tag=f"wsb{q}", name=f"wsb{q}")
                    self.wtsb = wp.tile([128, NS, D], bf16, tag=f"wtsb{q}", name=f"wtsb{q}")
                    # xts is dead after the y2 phase; park it in the slot
                    # of a decode-stage tile (otsb/zbf, allocated at the
                    # very end) so it costs no extra SBUF
                    self.xts = st.tile([128, NS, bl], bf16,
                                       tag="otsb" if q == "a" else "zbf",
                                       name=f"xts{q}")
                    self.dma_w = lambda: nc.sync.dma_start(
                        out=self.wsb, in_=w_d.rearrange("(t p) n -> p t n", p=128))
                    # per-s-tile DMAs so the first y2 matmuls only wait on
                    # the first 512KB slice, not the full 4MB transfer
                    self.dma_wt = lambda: [nc.sync.dma_start(
                        out=self.wtsb[:, s, :],
                        in_=wt_d.rearrange("(s p) e -> p s e", p=128)[:, s, :])
                        for s in range(NS)]
                    self.dma_x = lambda: nc.sync.dma_start(
                        out=self.xts, in_=xt_d.rearrange("(s p) b -> p s b", p=128))
                    self.y2 = st.tile([128, self.fd], bf16, tag=f"y2{q}", name=f"y2{q}")
                    self.xb = [st.tile([128, self.fd], fp32, tag=f"xb{i}{q}",
                                       name=f"xb{i}{q}") for i in range(2)]
                    self.uTs = [st.tile([128, NS * bl // 2], bf16,
                                        tag=f"uT{p}{q}", name=f"uT{p}{q}")
                                for p in range(2)]
                    self.xtmp = [st.tile([128, CHUNK], bf16, tag=f"xtmp{j}{q}",
                                         name=f"xtmp{j}{q}") for j in range(self.nch)]
                    self.pre = [st.tile([128, CHUNK], fp32, tag=f"pre{j}{q}",
                                        name=f"pre{j}{q}") for j in range(self.nch)]
                    nc.vector.memset(self.xb[0], 0.0)

                def y2_phase(self):
                    # s-major so the first matmul only needs the first
                    # per-s-tile slice of the wtsb DMA, not all 4MB
                    bl, tpc = self.bl, self.tpc
                    for j in range(self.nch):
                        py = ps_g.tile([128, CHUNK], fp32, tag="pg")
                        for s in range(NS):
                            for tt in range(tpc):
                                t = j * tpc + tt
                                nc.tensor.matmul(
                                    py[:, tt * bl:(tt + 1) * bl],
                                    self.wtsb[:, s, t * 128:(t + 1) * 128],
                                    self.xts[:, s, :],
                                    start=(s == 0 and tt == 0),
                                    stop=(s == NS - 1 and tt == tpc - 1))
                        nc.scalar.mul(
                            self.y2[:, j * CHUNK:(j + 1) * CHUNK], py, TAU)

                def mm_phase(self, k):
                    """u-phase + grad-phase + v-combine (v written in-place
                    into pre). Emitted t-major so u matmuls for d-chunk j
                    only wait on xtmp[j]."""
                    bl, tpc, sgp = self.bl, self.tpc, self.sgp
                    pus = [ps_u.tile([128, CHUNK], fp32, tag="pu",
                                     name=f"pu{self.q}{p}") for p in range(self.npu)]
                    for t in range(NT):
                        for s in range(NS):
                            nc.tensor.matmul(
                                pus[s // sgp][:, (s % sgp) * bl:(s % sgp + 1) * bl],
                                self.wsb[:, t, s * 128:(s + 1) * 128],
                                self.xtmp[t // tpc][:, (t % tpc) * bl:(t % tpc + 1) * bl],
                                start=(t == 0 and s % sgp == 0),
                                stop=(t == NT - 1 and s % sgp == sgp - 1))
                    # two half-drains on different engines run in
                    # parallel, halving the uT latency that gates grad;
                    # separate tiles split the dependency so grad's first
                    # s-chains start after the first half alone
                    half = NS * bl // 2
                    for h in range(2):
                        src_ap = (pus[h] if self.npu == 2
                                  else pus[0][:, h * half:(h + 1) * half])
                        if h == 0:
                            nc.scalar.copy(self.uTs[0], src_ap)
                        else:
                            nc.vector.tensor_copy(self.uTs[1], src_ap)
                    for j in range(self.nch):
                        pg = ps_g.tile([128, CHUNK], fp32, tag="pg")
                        for tt in range(tpc):
                            t = j * tpc + tt
                            for s in range(NS):
                                nc.tensor.matmul(
                                    pg[:, tt * bl:(tt + 1) * bl],
                                    self.wtsb[:, s, t * 128:(t + 1) * 128],
                                    self.uTs[s // (NS // 2)][
                                        :, (s % (NS // 2)) * bl:
                                        (s % (NS // 2) + 1) * bl],
                                    start=(tt == 0 and s == 0),
                                    stop=(tt == tpc - 1 and s == NS - 1))
                        # v = pre - TAU*grad, in place (pre is rebuilt below)
                        nc.vector.scalar_tensor_tensor(
                            self.pre[j], pg, -TAU, self.pre[j],
                            op0=OP.mult, op1=OP.add)

                def act_phase(self, k):
                    """Group soft-threshold + momentum on each chunk.
                    k == 1 reads v from y2; else v is in pre (in-place)."""
                    xnew, xold = self.xb[k % 2], self.xb[(k - 1) % 2]
                    m = ms[k - 1]
                    last = k == num_layers
                    for j in range(self.nch):
                        sl = slice(j * CHUNK, (j + 1) * CHUNK)
                        vj = self.y2[:, sl] if k == 1 else self.pre[j][:, :]
                        v2 = scr1.tile([128, CHUNK], bf16, tag="v2")
                        nc.scalar.square(v2, vj)
                        gs = ps_s.tile([128, CHUNK], fp32, tag="gs")
                        nc.tensor.matmul(gs, bmat, v2, start=True, stop=True)
                        nrm = scr.tile([128, CHUNK], fp32, tag="nrm")
                        # +1e-30 guards reciprocal_approx_fast's undefined
                        # 0-input; relu(1 - CTH/1e-15) = 0 matches reference
                        nc.scalar.activation(nrm, gs, AF.Sqrt,
                                             bias=eps[:, :], scale=1.0)
                        invn = scr.tile([128, CHUNK], fp32, tag="invn")
                        nc.vector.reciprocal_approx_fast(invn, nrm)
                        scl = scr.tile([128, CHUNK], fp32, tag="scl")
                        nc.scalar.activation(scl, invn, AF.Relu,
                                             bias=1.0, scale=-CTH)
                        # xnew = max(v, 0) * scl
                        nc.vector.scalar_tensor_tensor(
                            xnew[:, sl], vj, 0.0, scl,
                            op0=OP.max, op1=OP.mult)
                        if not last:
                            dd = scr3.tile([128, CHUNK], fp32, tag="dd")
                            nc.gpsimd.tensor_sub(dd, xnew[:, sl], xold[:, sl])
                            # pre = m*dd + xnew == fp32 xtmp; the bf16 copy
                            # (fast DVE tensor_copy, no extra stt) must read
                            # it before the in-place +y2 (Tile orders WAR)
                            nc.vector.scalar_tensor_tensor(
                                self.pre[j], dd, m, xnew[:, sl],
                                op0=OP.mult, op1=OP.add)
                            nc.vector.tensor_copy(self.xtmp[j], self.pre[j])
                            nc.gpsimd.tensor_add(self.pre[j], self.pre[j],
                                                 self.y2[:, sl])

                def decode(self):
                    bl, sgp = self.bl, self.sgp
                    z = self.xb[num_layers % 2]
                    zbf = st.tile([128, self.fd], bf16, tag="zbf")
                    nc.scalar.copy(zbf, z)
                    pds = [ps_u.tile([128, CHUNK], fp32, tag="pu",
                                     name=f"pd{self.q}{p}") for p in range(self.npu)]
                    for t in range(NT):
                        for s in range(NS):
                            nc.tensor.matmul(
                                pds[s // sgp][:, (s % sgp) * bl:(s % sgp + 1) * bl],
                                self.wsb[:, t, s * 128:(s + 1) * 128],
                                zbf[:, t * bl:(t + 1) * bl],
                                start=(t == 0 and s % sgp == 0),
                                stop=(t == NT - 1 and s % sgp == sgp - 1))
                    otsb = st.tile([128, NS, bl], fp32, tag="otsb")
                    for s in range(NS):
                        nc.scalar.copy(
                            otsb[:, s, :],
                            pds[s // sgp][:, (s % sgp) * bl:(s % sgp + 1) * bl])
                    nc.sync.dma_start(
                        out=self.ot_d.rearrange("(s p) b -> p s b", p=128),
                        in_=otsb)

            A = Sub("a", BLA, xta, wa, wta, ota)
            Bs = Sub("b", BLB, xtb, wb, wtb, otb)
            nc.sync.dma_start(out=bmat, in_=bm[:, :])
            A.dma_x(); A.dma_wt()
            Bs.dma_x(); Bs.dma_wt()
            A.dma_w(); Bs.dma_w()

            A.y2_phase(); Bs.y2_phase()
            A.act_phase(1); Bs.act_phase(1)
            # B's act is skewed half an iteration behind A's so its gs
            # matmuls never head the PE stream right behind B-grad (the
            # act chain then has A's matmul span to complete instead).
            A.mm_phase(2); Bs.mm_phase(2); A.act_phase(2)
            for k in range(3, num_layers + 1):
                A.mm_phase(k)
                Bs.act_phase(k - 1)
                Bs.mm_phase(k)
                A.act_phase(k)
            Bs.act_phase(num_layers)
            A.decode(); Bs.decode()

    nc.compile()
    return nc


_CACHED = {}


def _get_nc(num_layers=NUM_LAYERS):
    if num_layers not in _CACHED:
        _CACHED[num_layers] = build(num_layers)
    return _CACHED[num_layers]


def make_in_maps(x, w):
    """x [B,C,N] fp32, w [C,D,N] fp32 -> list of 8 per-core input dicts."""
    import ml_dtypes

    bf = ml_dtypes.bfloat16
    x = np.asarray(x, dtype=np.float32)
    w32 = np.ascontiguousarray(np.asarray(w, dtype=np.float32))
    wb_ = w32.astype(bf)
    wtb_ = np.ascontiguousarray(w32.transpose(0, 2, 1)).astype(bf)
    bmb = _bmat_np().astype(bf)
    maps = []
    for i in range(N_CORES):
        ca = 0 if i < 4 else 1
        ra = (i % 4) * BLA
        xa = np.ascontiguousarray(x[ra:ra + BLA, ca].T).astype(bf)  # [N, BLA]
        rb = i * BLB
        xb_ = np.ascontiguousarray(x[rb:rb + BLB, 2].T).astype(bf)  # [N, BLB]
        maps.append({
            "xta": xa, "wa": wb_[ca], "wta": wtb_[ca],
            "xtb": xb_, "wb": wb_[2], "wtb": wtb_[2], "bm": bmb,
        })
    return maps


def assemble_out(results):
    out = np.empty((B, C, N), np.float32)
    for i in range(N_CORES):
        ca = 0 if i < 4 else 1
        ra = (i % 4) * BLA
        out[ra:ra + BLA, ca] = results[i]["ota"].T  # [N, BLA] -> [BLA, N]
        rb = i * BLB
        out[rb:rb + BLB, 2] = results[i]["otb"].T
    return out


def kernel(x, W):
    from concourse.bass_utils import run_bass_kernel_spmd

    nc = _get_nc()
    res = run_bass_kernel_spmd(nc, make_in_maps(x, W), list(range(N_CORES)))
    return assemble_out(res.results)


if __name__ == "__main__":
    xs = np.random.randn(B, C, N).astype(np.float32)
    ws = np.random.randn(C, D, N).astype(np.float32)
    ws /= np.linalg.norm(ws, axis=-1, keepdims=True)
    out = kernel(xs, ws)
    print("out", out.shape, out.dtype, float(np.abs(out).mean()))
